# revision 13
# baseline (speedup 1.0000x reference)
"""Bass/Trainium2 kernel for the 4-layer per-node-weight GCN (SMPL 24-joint tree).

Strategy: fold each layer's per-node linear + adjacency-weighted aggregation
into a single block-sparse matmul with host-precomputed combined weights
  Wc_l[(n,i),(m,o)] = (adjw_l * adj)[m,n] * w_l[n,i,o]
so that  h_{l+1}[b,(m,o)] = act( h_l[b,(n,i)] @ Wc_l + b_l[o] ).
On device the data lives as [features(partitions), batch(free)] and each layer
is a K x M tiling of 128x128 blocks, skipping all-zero blocks (tree adjacency
=> ~1/3 of blocks are nonzero). All nonzero blocks are packed host-side into
one [128, total] tensor loaded with a single DMA. Batch is sharded across the
8 cores.
"""

import numpy as np

import concourse.mybir as mybir
from concourse import bacc
from concourse.tile import TileContext
from concourse import bass_utils

JOINT_TREES = [0, 0, 0, 0, 1, 2, 3, 4, 5, 6, 7, 8, 9, 9, 9, 12, 13, 14, 16, 17, 18, 19, 20, 21]
NN = 24
B = 65536
FIN = 7
WID = 64
OUT = 13
NCORES = 8
BLOC = B // NCORES          # 8192 samples per core
NCHUNK = 512                # batch columns per matmul pass (fp32 PSUM bank)
NCHUNKS = BLOC // NCHUNK    # 16

K_IN = NN * FIN             # 168
M_MID = NN * WID            # 1536
M_OUT = NN * OUT            # 312

Ks = [K_IN, M_MID, M_MID, M_MID]
Ms = [M_MID, M_MID, M_MID, M_OUT]
MT = [128, 128, 128, 104]   # M tile sizes (104 = 8 nodes * 13 for layer 3)

F32 = mybir.dt.float32


def _build_adj():
    A = np.eye(NN, dtype=np.float32)
    for i, j in enumerate(JOINT_TREES):
        if i != j:
            A[j, i] = 1.0
            A[i, j] = 1.0
    return A


def _combined_weights(w0, w1, w2, w3, adjw0, adjw1, adjw2, adjw3):
    adj = _build_adj()
    out = []
    for w, aw in ((w0, adjw0), (w1, adjw1), (w2, adjw2), (w3, adjw3)):
        A = (np.asarray(aw, np.float32) * adj).astype(np.float32)   # [m, n]
        # Wc[(n,i),(m,o)] = A[m,n] * w[n,i,o]
        Wc = np.einsum("mn,nio->nimo", A, np.asarray(w, np.float32))
        n, i, m, o = Wc.shape
        out.append(np.ascontiguousarray(Wc.reshape(n * i, m * o), dtype=np.float32))
    # root input mask: x[:,0,:] is zeroed before layer 0 == zero source rows n=0
    out[0][0:FIN, :] = 0.0
    return out


def _tiles(total, tsz):
    return [(t0, min(tsz, total - t0)) for t0 in range(0, total, tsz)]


def _layout():
    """Static block layout: per layer, the (ki, mi) nonzero blocks and their
    column offset in the packed weight tensor. Sparsity comes from the fixed
    tree adjacency, so it is known without the numeric weights."""
    adj = _build_adj()
    ktiles = [_tiles(Ks[l], 128) for l in range(4)]
    mtiles = [_tiles(Ms[l], MT[l]) for l in range(4)]
    feat = [FIN, WID, WID, WID]   # input feature width per layer
    ofeat = [WID, WID, WID, OUT]

    blocks = []     # per layer: list of (ki, mi, k0, kh, m0, mw, col_off)
    off = 0
    for l in range(4):
        bl = []
        for mi, (m0, mw) in enumerate(mtiles[l]):
            for ki, (k0, kh) in enumerate(ktiles[l]):
                # nodes covered by this K/M tile
                src = set(range(k0 // feat[l], (k0 + kh - 1) // feat[l] + 1))
                dst = set(range(m0 // ofeat[l], (m0 + mw - 1) // ofeat[l] + 1))
                if any(adj[m, n] for n in src for m in dst):
                    bl.append((ki, mi, k0, kh, m0, mw, off))
                    off += mw
        blocks.append(bl)
    return ktiles, mtiles, blocks, off


_CACHE = {}


def _build_program():
    ktiles, mtiles, blocks, wtot = _layout()
    nbias = sum(len(mtiles[l]) for l in range(4))

    nc = bacc.Bacc()
    xt = nc.dram_tensor("xt", [K_IN, BLOC], F32, kind="ExternalInput")
    wdr = nc.dram_tensor("wpack", [128, wtot], F32, kind="ExternalInput")
    bdr = nc.dram_tensor("bpack", [128, nbias], F32, kind="ExternalInput")
    out = nc.dram_tensor("out", [M_OUT, BLOC], F32, kind="ExternalOutput")

    Relu = mybir.ActivationFunctionType.Relu
    Ident = mybir.ActivationFunctionType.Identity

    with TileContext(nc) as tc:
        with (
            tc.tile_pool(name="wp", bufs=1) as wp,
            tc.tile_pool(name="xp", bufs=2) as xp,
            tc.tile_pool(name="hp", bufs=1) as hp,
            tc.tile_pool(name="op", bufs=2) as op_,
            tc.tile_pool(name="pp", bufs=4, space="PSUM") as pp,
        ):
            wtile = wp.tile([128, wtot], F32, tag="wpack")
            nc.sync.dma_start(wtile[:, :], wdr[:, :])
            btile = wp.tile([128, nbias], F32, tag="bpack")
            nc.sync.dma_start(btile[:, :], bdr[:, :])

            for c in range(NCHUNKS):
                c0 = c * NCHUNK
                xa = xp.tile([128, NCHUNK], F32, tag="xa")
                nc.sync.dma_start(xa[:, :], xt[0:128, c0:c0 + NCHUNK])
                xb = xp.tile([K_IN - 128, NCHUNK], F32, tag="xb")
                nc.sync.dma_start(xb[:, :], xt[128:K_IN, c0:c0 + NCHUNK])
                prev = [xa[:, :], xb[:, :]]

                bias_idx = 0
                for l in range(4):
                    cur = []
                    for mi, (m0, mw) in enumerate(mtiles[l]):
                        blk = [b for b in blocks[l] if b[1] == mi]
                        ps = pp.tile([mw, NCHUNK], F32, tag="ps")
                        for j, (ki, _, k0, kh, _, _, off) in enumerate(blk):
                            nc.tensor.matmul(
                                ps[:, :],
                                wtile[0:kh, off:off + mw],
                                prev[ki],
                                start=(j == 0),
                                stop=(j == len(blk) - 1),
                            )
                        bias_ap = btile[0:mw, bias_idx:bias_idx + 1]
                        bias_idx += 1
                        if l < 3:
                            h = hp.tile([mw, NCHUNK], F32, tag=f"h{l}_{mi}")
                            nc.scalar.activation(h[:, :], ps[:, :], Relu,
                                                 bias=bias_ap)
                            cur.append(h[:, :])
                        else:
                            o = op_.tile([mw, NCHUNK], F32, tag=f"o{mi}")
                            nc.scalar.activation(o[:, :], ps[:, :], Ident,
                                                 bias=bias_ap)
                            nc.sync.dma_start(out[m0:m0 + mw, c0:c0 + NCHUNK],
                                              o[:, :])
                    prev = cur
    if not nc.is_finalized():
        nc.finalize()
    return nc


def _pack_inputs(x, w0, w1, w2, w3, adjw0, adjw1, adjw2, adjw3, b0, b1, b2, b3):
    """Host-side packing: combined weights -> wpack, biases -> bpack, x -> xt."""
    wcs = _combined_weights(w0, w1, w2, w3, adjw0, adjw1, adjw2, adjw3)
    ktiles, mtiles, blocks, wtot = _layout()
    wpack = np.zeros((128, wtot), dtype=np.float32)
    for l in range(4):
        for (ki, mi, k0, kh, m0, mw, off) in blocks[l]:
            wpack[0:kh, off:off + mw] = wcs[l][k0:k0 + kh, m0:m0 + mw]

    nbias = sum(len(mtiles[l]) for l in range(4))
    bpack = np.zeros((128, nbias), dtype=np.float32)
    bs = [np.tile(np.asarray(b, np.float32), NN) for b in (b0, b1, b2, b3)]
    bi = 0
    for l in range(4):
        for (m0, mw) in mtiles[l]:
            bpack[0:mw, bi] = bs[l][m0:m0 + mw]
            bi += 1

    xt = np.asarray(x, np.float32).reshape(B, K_IN).T
    in_maps = []
    for c in range(NCORES):
        in_maps.append({
            "xt": np.ascontiguousarray(xt[:, c * BLOC:(c + 1) * BLOC]),
            "wpack": wpack,
            "bpack": bpack,
        })
    return in_maps


def _get_nc():
    if "nc" not in _CACHE:
        _CACHE["nc"] = _build_program()
    return _CACHE["nc"]


def kernel(x, w0, w1, w2, w3, adjw0, adjw1, adjw2, adjw3, b0, b1, b2, b3):
    in_maps = _pack_inputs(x, w0, w1, w2, w3, adjw0, adjw1, adjw2, adjw3,
                           b0, b1, b2, b3)
    nc = _get_nc()
    res = bass_utils.run_bass_kernel_spmd(nc, in_maps, core_ids=list(range(NCORES)))
    outs = [r["out"] for r in res.results]          # each [312, 8192]
    out_t = np.concatenate(outs, axis=1)            # [312, B]
    return np.ascontiguousarray(
        out_t.reshape(NN, OUT, B).transpose(2, 0, 1))
